# revision 11
# baseline (speedup 1.0000x reference)
"""Trainium2 Bass kernel for nn_NeuralNetwork_86990267613505 (topk_masking).

Network (per reference):
  cx = sigmoid(tanh(input @ W_c1.T + b_c1) @ W_c2.T)          # [B] gate
  x  = kwta(input @ W1.T + b1, k=int(cx*1024))                # [B,1024]
  x  = kwta(x @ W2.T + b2,     k=int(cx*512))                 # [B,512]
  x  = kwta(x @ W3.T + b3,     k=int(cx*1024))                # [B,1024]
  out = x @ W4.T                                              # [B,1024]

Sharding: the two big matmuls (contraction over S2=32768) are column-sharded
over the contraction dim across 8 cores (4096 each); partial sums are combined
with a single fused ReduceScatter of [B, 512+1024] which also distributes the
batch (32 rows per core).  Everything after is data-parallel per core.

Phase A streams bf16 hi/lo pairs (3 matmul passes: hi*hi + hi*lo + lo*hi) in
b-tile-major order so consecutive matmuls share the stationary operand (the
hardware fast-paths repeated weight loads).  Biases are applied after the
ReduceScatter.

kwta: per-row exact k-th-largest via radix-5 bisection with a constant width
schedule: each pass is count -> is_gt -> PE-matmul replica-reduce -> one
activation producing the next probes (update biases precomputed off the
critical path).  Then band extraction + two max8 passes + indicator-pick,
and mask = (x >= thresh) * x.
"""

import numpy as np

import concourse.bacc as bacc
import concourse.mybir as mybir
import concourse.tile as tile
from concourse import bass_utils

F32 = mybir.dt.float32
BF16 = mybir.dt.bfloat16
I32 = mybir.dt.int32
ALU = mybir.AluOpType
ACTF = mybir.ActivationFunctionType

HID = 512
N1 = 2 * HID      # 1024
N3 = 1024         # HEADS
R = 32            # rows per core after scatter
C = 4             # partition replication for probing
BIG = 1e30
N_PASS = 6        # radix-5 bisection passes
W0 = 32.0         # initial bisection width, interval [-16, 16)


def widths():
    w = [W0]
    for _ in range(N_PASS + 3):
        w.append(w[-1] / 5.0)
    return w


class Cfg:
    def __init__(self, S2=32768, B=256, NC=8, chunk=4, debug=False):
        assert B // NC == R
        self.S2, self.B, self.NC, self.chunk = S2, B, NC, chunk
        self.debug = debug
        self.no_collective = False
        self.loop_n = 0
        self.phase = None  # None | 'notail'
        self.KSH = S2 // NC            # contraction shard per core
        self.KT = self.KSH // 128      # k-tiles
        assert self.KT % chunk == 0
        self.SW = B + 3 * HID          # stream free width per k-tile
        self.b_tiles = [(s, min(128, B - s)) for s in range(0, B, 128)]


def _floorize(nc, sb, val_ap, name):
    """floor(val) for val >= 0, given HW float->int casts are RNE."""
    ki = sb.tile([128, 1], I32, name=f"{name}_i")
    kb = sb.tile([128, 1], F32, name=f"{name}_b")
    cmp = sb.tile([128, 1], F32, name=f"{name}_c")
    kf = sb.tile([128, 1], F32, name=f"{name}_f")
    nc.vector.tensor_copy(ki[:], val_ap)
    nc.vector.tensor_copy(kb[:], ki[:])
    nc.vector.tensor_tensor(cmp[:], kb[:], val_ap, ALU.is_gt)
    nc.vector.tensor_sub(kf[:], kb[:], cmp[:])
    return kf


def _floorize2(nc, sb, val_ap, name):
    """floor(val) columnwise for val >= 0 on a [128,2] tile (RNE casts)."""
    ki = sb.tile([128, 2], I32, name=f"{name}_i")
    kb = sb.tile([128, 2], F32, name=f"{name}_b")
    cmp = sb.tile([128, 2], F32, name=f"{name}_c")
    kf = sb.tile([128, 2], F32, name=f"{name}_f")
    nc.vector.tensor_copy(ki[:], val_ap)
    nc.vector.tensor_copy(kb[:], ki[:])
    nc.vector.tensor_tensor(cmp[:], kb[:], val_ap, ALU.is_gt)
    nc.vector.tensor_sub(kf[:], kb[:], cmp[:])
    return kf


def _kwta(nc, sb, ps, xb, x_r, kv, kv_r, kf_r, n, consts, lname,
          skip_pass0=False):
    """xb: [128, n] bf16 (rows replicated 4x: partition 32c+r = row r).
    x_r: [R, n] f32 AP of the true values.  kv: [128,1] f32 = cx*n - 1.
    kf: [R-usable ,1] f32 = floor(cx*n).  Returns masked [R, n] f32 tile.

    Radix-5 bisection: probes_p = lo_p + w_p*frac (frac=(c+1)/5 per replica),
    j = #probes with count >= k, lo_{p+1} = lo_p + w_{p+1}*j.  The probe
    update is one activation: probes_{p+1} = w_{p+1}*j + (lo_p + w_{p+1}*frac),
    with the bias terms maintained off the critical path."""
    frac, iota16, bis0 = consts["frac"], consts["iota16"], consts["bis0"]
    W = consts["widths"]

    # rolling state: probes / lofrac / lo   (all [128,1] f32)
    probes = bis0[:, 0:1]
    lofrac = bis0[:, 1:2]
    lo = bis0[:, 2:3]

    trash = sb.tile([128, n], BF16, tag="kw_tr", name=f"{lname}_tr")
    # band-value buffer: memset early, off the critical path
    bandv = sb.tile([R, n], F32, tag="kw_bv", name=f"{lname}_bv")
    nc.vector.memset(bandv[:], -BIG)
    thr = sb.tile([R, 1], F32, name=f"{lname}_thr")
    nc.vector.memset(thr[:], BIG)

    lo5h = None
    for p in range(N_PASS):
        cnt = sb.tile([128, 1], F32, tag="kw_cnt", bufs=2, name=f"{lname}_cnt{p}")
        if not (p == 0 and skip_pass0):
            nc.vector.tensor_scalar(
                trash[:], xb[:], probes[:, 0:1], None, ALU.is_ge, ALU.add,
                accum_out=cnt[:],
            )
        else:
            cnt = consts["pass0_cnt"]
        ge = sb.tile([128, 1], F32, tag="kw_ge", bufs=2, name=f"{lname}_ge{p}")
        nc.vector.tensor_scalar(ge[:], cnt[:], kv, None, ALU.is_gt)
        # all-DVE replica-reduce (cross-engine hops are ~1us each; avoid)
        s64 = sb.tile([64, 1], F32, tag="kw_s64", bufs=2, name=f"{lname}_s64_{p}")
        j2 = sb.tile([64, 1], F32, tag="kw_j2", bufs=2, name=f"{lname}_j2_{p}")
        s32 = sb.tile([32, 1], F32, tag="kw_s32", bufs=2, name=f"{lname}_s32_{p}")
        jall = sb.tile([128, 1], F32, tag="kw_j", bufs=2, name=f"{lname}_j{p}")
        nc.vector.tensor_copy(s64[:], ge[64:128, :])
        nc.vector.tensor_add(j2[:], ge[0:64, :], s64[:])
        nc.vector.tensor_copy(s32[:], j2[32:64, :])
        nc.vector.tensor_add(jall[0:32, :], j2[0:32, :], s32[:])
        nc.vector.tensor_copy(jall[32:64, :], jall[0:32, :])
        nc.vector.tensor_copy(jall[64:128, :], jall[0:64, :])
        if p < N_PASS - 1:
            pn = sb.tile([128, 1], F32, tag="kw_pr", bufs=2, name=f"{lname}_pr{p+1}")
            nc.vector.scalar_tensor_tensor(
                pn[:], jall[:], W[p + 1], lofrac[:, 0:1], ALU.mult, ALU.add)
            lon = sb.tile([128, 1], F32, tag="kw_lo", bufs=2, name=f"{lname}_lo{p+1}")
            nc.vector.scalar_tensor_tensor(
                lon[:], jall[:], W[p + 1], lo[:, 0:1], ALU.mult, ALU.add)
            lfn = sb.tile([128, 1], F32, tag="kw_lf", bufs=2, name=f"{lname}_lf{p+1}")
            nc.vector.scalar_tensor_tensor(
                lfn[:], frac[:], W[p + 2], lon[:, 0:1], ALU.mult, ALU.add)
            probes, lofrac, lo = pn, lfn, lon
            if p == N_PASS - 2:
                # lo5h = lo_5 + w_6 (bias for hi_6), off critical path
                lo5h = sb.tile([128, 1], F32, name=f"{lname}_lo5h")
                nc.vector.tensor_scalar(lo5h[:], lon[:], W[N_PASS], None, ALU.add)
        else:
            lo6 = sb.tile([128, 1], F32, tag="kw_lo", bufs=2, name=f"{lname}_lo6")
            nc.vector.scalar_tensor_tensor(
                lo6[:], jall[:], W[N_PASS], lo[:, 0:1], ALU.mult, ALU.add)
            hi6 = sb.tile([128, 1], F32, name=f"{lname}_hi6")
            nc.vector.scalar_tensor_tensor(
                hi6[:], jall[:], W[N_PASS], lo5h[:, 0:1], ALU.mult, ALU.add)
            lo = lo6

    # chi = count(x >= hi)
    chi = sb.tile([128, 1], F32, name=f"{lname}_chi")
    nc.vector.tensor_scalar(
        trash[:], xb[:], hi6[:, 0:1], None, ALU.is_ge, ALU.add, accum_out=chi[:],
    )
    # band on rows 0:R: x in [lo, hi) else -BIG
    bhi = sb.tile([R, n], BF16, tag="kw_bhi", name=f"{lname}_bhi")
    binb = sb.tile([R, n], I32, tag="kw_binb", name=f"{lname}_binb")
    nc.vector.tensor_scalar(bhi[:], xb[0:R, :], hi6[0:R, 0:1], None, ALU.is_lt)
    nc.vector.scalar_tensor_tensor(
        binb[:], xb[0:R, :], lo[0:R, 0:1], bhi[:], ALU.is_ge, ALU.mult)
    nc.vector.copy_predicated(bandv[:], binb[:], x_r)
    # top-16 of band
    m16 = sb.tile([R, 16], F32, name=f"{lname}_m16")
    band2 = sb.tile([R, n], F32, tag="kw_b2", name=f"{lname}_b2")
    nc.vector.max(m16[:, 0:8], bandv[:])
    nc.vector.match_replace(band2[:], m16[:, 0:8], bandv[:], -BIG)
    nc.vector.max(m16[:, 8:16], band2[:])
    # pick (k - chi - 1)-th
    rf = sb.tile([R, 1], F32, name=f"{lname}_rf")
    nc.vector.tensor_sub(rf[:], kf_r, chi[0:R, :])
    nc.vector.tensor_scalar(rf[:], rf[:], 1.0, None, ALU.subtract)
    nc.vector.tensor_scalar(rf[:], rf[:], 0.0, 15.0, ALU.max, ALU.min)
    ind = sb.tile([R, 16], F32, name=f"{lname}_ind")
    nc.vector.tensor_scalar(ind[:], iota16[0:R, :], rf[:, 0:1], None, ALU.is_equal)
    iv = sb.tile([R, 16], F32, name=f"{lname}_iv")
    nc.vector.tensor_mul(iv[:], ind[:], m16[:])
    vk = sb.tile([R, 1], F32, name=f"{lname}_vk")
    nc.vector.reduce_sum(vk[:], iv[:], axis=mybir.AxisListType.X)
    # thr = k>=1 ? vk : +BIG   (k >= 1  <=>  kv = cx*n-1 >= 0)
    gint = sb.tile([R, 1], I32, name=f"{lname}_g")
    nc.vector.tensor_scalar(gint[:], kv_r, 0.0, None, ALU.is_ge)
    nc.vector.copy_predicated(thr[:], gint[:], vk[:])
    # masked = (x >= thresh) * x
    masked = sb.tile([R, n], F32, tag="kw_mask", name=f"{lname}_masked")
    nc.vector.scalar_tensor_tensor(
        masked[:], x_r, thr[:, 0:1], x_r, ALU.is_ge, ALU.mult)
    return masked, thr


def _transpose_chunks(nc, sb, pst, masked, n, ident, rep, lname):
    """masked [R, n] f32 -> list of xT tiles: [128, C*R] (rep) or [128, R]."""
    pts = []
    for ch in range(n // 128):
        pt = pst.tile([128, R], F32, tag="tp", name=f"{lname}_pt{ch}")
        nc.tensor.transpose(pt[:], masked[:, 128 * ch:128 * (ch + 1)],
                            ident[0:R, 0:R])
        pts.append(pt)
    tiles = []
    for ch, pt in enumerate(pts):
        if rep:
            xt = sb.tile([128, C * R], F32, tag="kw_xt", bufs=8,
                         name=f"{lname}_xt{ch}")
            nc.vector.tensor_copy(
                xt[:].rearrange("p (c r) -> p c r", c=C),
                pt[:, :].unsqueeze(1).broadcast_to([128, C, R]),
            )
        else:
            xt = sb.tile([128, R], F32, tag="kw_xt", bufs=8,
                         name=f"{lname}_xt{ch}")
            nc.vector.tensor_copy(xt[:], pt[:])
        tiles.append(xt)
    return tiles


def build_nc(cfg: Cfg):
    nc = bacc.Bacc("TRN2", target_bir_lowering=False, debug=False,
                   num_devices=cfg.NC)
    B, NC, KT, SW, chunk = cfg.B, cfg.NC, cfg.KT, cfg.SW, cfg.chunk
    W = widths()

    stream_d = nc.dram_tensor("stream", [KT, 128, 2, SW], BF16, kind="ExternalInput")
    ident_d = nc.dram_tensor("ident", [128, 128], F32, kind="ExternalInput")
    bc1rep_d = nc.dram_tensor("bc1rep", [128, HID], F32, kind="ExternalInput")
    b1rep_d = nc.dram_tensor("b1rep", [128, N1], F32, kind="ExternalInput")
    b2rep_d = nc.dram_tensor("b2rep", [128, HID], F32, kind="ExternalInput")
    b3rep_d = nc.dram_tensor("b3rep", [128, N3], F32, kind="ExternalInput")
    wc2rep_d = nc.dram_tensor("wc2rep", [128, HID], F32, kind="ExternalInput")
    frac_d = nc.dram_tensor("frac", [128, 1], F32, kind="ExternalInput")
    bis0_d = nc.dram_tensor("bis0", [128, 3], F32, kind="ExternalInput")
    nvec_d = nc.dram_tensor("nvec", [128, 2], F32, kind="ExternalInput")
    iota16_d = nc.dram_tensor("iota16", [R, 16], F32, kind="ExternalInput")
    w2t_d = nc.dram_tensor("w2t", [N1, HID], F32, kind="ExternalInput")
    w3t_d = nc.dram_tensor("w3t", [HID, N3], F32, kind="ExternalInput")
    w4t_d = nc.dram_tensor("w4t", [N3, N3], F32, kind="ExternalInput")
    out_d = nc.dram_tensor("out", [R, N3], F32, kind="ExternalOutput")

    import contextlib
    with tile.TileContext(nc) as tc:
        loop_ctx = tc.For_i(0, cfg.loop_n, 1) if cfg.loop_n else contextlib.nullcontext()
        with (
            loop_ctx,
            tc.tile_pool(name="consts", bufs=1) as cp,
            tc.tile_pool(name="stream", bufs=2) as sp,
            tc.tile_pool(name="acc", bufs=1, space="PSUM") as ap,
            tc.tile_pool(name="sb", bufs=1) as sb,
            tc.tile_pool(name="pst", bufs=2, space="PSUM") as pst,
            tc.tile_pool(name="dram", bufs=1, space="DRAM") as dram,
        ):
            # ---- constants ----
            ident = cp.tile([128, 128], F32, name="ident")
            bc1rep = cp.tile([128, HID], F32, name="bc1rep")
            b1rep = cp.tile([128, N1], F32, name="b1rep")
            b2rep = cp.tile([128, HID], F32, name="b2rep")
            b3rep = cp.tile([128, N3], F32, name="b3rep")
            wc2rep = cp.tile([128, HID], F32, name="wc2rep")
            frac = cp.tile([128, 1], F32, name="frac")
            bis0 = cp.tile([128, 3], F32, name="bis0")
            nvec = cp.tile([128, 2], F32, name="nvec")
            iota16 = cp.tile([R, 16], F32, name="iota16")
            nc.sync.dma_start(ident[:], ident_d.ap())
            nc.sync.dma_start(bc1rep[:], bc1rep_d.ap())
            nc.sync.dma_start(b1rep[:], b1rep_d.ap())
            nc.sync.dma_start(b2rep[:], b2rep_d.ap())
            nc.sync.dma_start(b3rep[:], b3rep_d.ap())
            nc.sync.dma_start(wc2rep[:], wc2rep_d.ap())
            nc.sync.dma_start(frac[:], frac_d.ap())
            nc.sync.dma_start(bis0[:], bis0_d.ap())
            nc.sync.dma_start(nvec[:], nvec_d.ap())
            nc.sync.dma_start(iota16[:], iota16_d.ap())
            consts = {"ident": ident, "frac": frac, "iota16": iota16,
                      "bis0": bis0, "widths": W}

            # ---- phase A: streamed big matmuls ----
            pacc = {}
            for bi, (bs, bsz) in enumerate(cfg.b_tiles):
                pacc[bi] = ap.tile([bsz, 3 * HID], F32, tag="acc3", bufs=2,
                                   name=f"pacc_{bi}")

            n_chunks = KT // chunk
            for cki in range(n_chunks):
                st = sp.tile([128, chunk * 2 * SW], BF16, tag="st", name=f"st{cki}")
                src = stream_d.ap()[chunk * cki: chunk * (cki + 1)]
                nc.sync.dma_start(
                    st[:].rearrange("p (c t w) -> p c t w", c=chunk, t=2),
                    src.transpose([1, 0, 2, 3]),
                )
                for ki in range(chunk):
                    kt = chunk * cki + ki
                    hi = st[:, (2 * ki) * SW:(2 * ki + 1) * SW]
                    lo = st[:, (2 * ki + 1) * SW:(2 * ki + 2) * SW]
                    first, last = kt == 0, kt == KT - 1
                    # b-tile-major, lhsT bound once: passes (hi,hi) and (hi,lo)
                    # share the stationary hi-x chunk (fast-path weight reload).
                    for bi, (bs, bsz) in enumerate(cfg.b_tiles):
                        hx = hi[:, bs:bs + bsz]
                        lx = lo[:, bs:bs + bsz]
                        for pi, (lhsT, wb) in enumerate(
                                ((hx, hi), (hx, lo), (lx, hi))):
                            f = first and pi == 0
                            l = last and pi == 2
                            for o in range(3):
                                nc.tensor.matmul(
                                    pacc[bi][:, HID * o:HID * (o + 1)], lhsT,
                                    wb[:, B + HID * o:B + HID * (o + 1)],
                                    start=f, stop=l)

            # ---- tail weights (after the stream so they don't delay it) ----
            w2sb = cp.tile([128, 8 * HID], F32, name="w2sb")
            w3sb = cp.tile([128, 4 * N3], F32, name="w3sb")
            w4sb = cp.tile([128, 8 * N3], F32, name="w4sb")
            nc.sync.dma_start(
                w2sb[:].rearrange("p (c w) -> p c w", c=8),
                w2t_d.ap().rearrange("(c p) w -> p c w", p=128))
            nc.sync.dma_start(
                w3sb[:].rearrange("p (c w) -> p c w", c=4),
                w3t_d.ap().rearrange("(c p) w -> p c w", p=128))
            nc.sync.dma_start(
                w4sb[:].rearrange("p (c w) -> p c w", c=8),
                w4t_d.ap().rearrange("(c p) w -> p c w", p=128))

            # ---- phase B: PSUM -> DRAM, then ReduceScatter (bias added later)
            rs_in = dram.tile([B, 3 * HID], F32, name="rs_in")
            rs_out = dram.tile([R, 3 * HID], F32, name="rs_out")
            for bi, (bs, bsz) in enumerate(cfg.b_tiles):
                so = sb.tile([bsz, 3 * HID], F32, tag="rsin_sb", bufs=2,
                             name=f"so{bi}")
                nc.vector.tensor_copy(so[:], pacc[bi][:])
                nc.sync.dma_start(rs_in[bs:bs + bsz, :], so[:])
            if cfg.no_collective:
                nc.sync.dma_start(rs_out[:], rs_in[0:R, :])
            else:
                nc.gpsimd.collective_compute(
                    "ReduceScatter", ALU.add,
                    replica_groups=[list(range(NC))],
                    ins=[rs_in.opt()], outs=[rs_out.opt()],
                )

            # ---- phase C: replicated load + bias + gate ----
            xall = sb.tile([128, 3 * HID], F32, name="xall")
            for c in range(C):
                nc.sync.dma_start(xall[c * R:(c + 1) * R, :], rs_out[:])
            # xb1 = bf16(x1 + b1): bias fused into the bf16 copy for counting
            xb1 = sb.tile([128, N1], BF16, name="xb1")
            nc.vector.tensor_tensor(xb1[:], xall[:, HID:3 * HID], b1rep[:], ALU.add)
            # L1 pass-0 count: probes are constants, runs before the gate ends
            p0cnt = sb.tile([128, 1], F32, name="p0cnt")
            trash0 = sb.tile([128, N1], BF16, name="trash0")
            nc.vector.tensor_scalar(
                trash0[:], xb1[:], bis0[:, 0:1], None, ALU.is_ge, ALU.add,
                accum_out=p0cnt[:],
            )
            # gate: cx = sigmoid(tanh(xc1) @ wc2)
            xc1f = sb.tile([128, HID], F32, name="xc1f")
            nc.vector.tensor_tensor(xc1f[:], xall[:, 0:HID], bc1rep[:], ALU.add)
            th = sb.tile([128, HID], F32, name="tanh")
            nc.scalar.activation(th[:], xc1f[:], ACTF.Tanh)
            ztr = sb.tile([128, HID], F32, name="ztr")
            z = sb.tile([128, 1], F32, name="z")
            nc.vector.tensor_mul(ztr[:], th[:], wc2rep[:])
            nc.vector.reduce_sum(z[:], ztr[:], axis=mybir.AxisListType.X)
            ez = sb.tile([128, 1], F32, name="ez")
            nc.scalar.activation(ez[:], z[:], ACTF.Exp, scale=-1.0)
            ez1 = sb.tile([128, 1], F32, name="ez1")
            nc.vector.tensor_scalar(ez1[:], ez[:], 1.0, None, ALU.add)
            cx = sb.tile([128, 1], F32, name="cx")
            nc.vector.reciprocal(cx[:], ez1[:])
            # v12 = cx*[1024,512]; kv = v-1  (cnt > kv <=> cnt >= floor(cx*n))
            v12 = sb.tile([128, 2], F32, name="v12")
            nc.vector.tensor_tensor(v12[:], cx[:].broadcast_to([128, 2]),
                                    nvec[:], ALU.mult)
            kv12 = sb.tile([128, 2], F32, name="kv12")
            nc.vector.tensor_scalar(kv12[:], v12[:], 1.0, None, ALU.subtract)

            do_tail = cfg.phase != "notail"
            if not do_tail:
                nt = sb.tile([R, N3], F32, name="nt")
                nc.vector.tensor_copy(nt[:], xall[0:R, HID:3 * HID])
                nc.vector.tensor_add(nt[:, 0:1], w2sb[0:R, 0:1], w3sb[0:R, 0:1])
                nc.vector.tensor_add(nt[:, 1:2], w4sb[0:R, 0:1], cx[0:R, :])
                nc.vector.tensor_add(nt[:, 2:3], kv12[0:R, 0:1], kv12[0:R, 1:2])
                nc.vector.tensor_add(nt[:, 3:4], xb1[0:R, 0:1], frac[0:R, :])
                nc.vector.tensor_add(nt[:, 4:5], ident[0:R, 0:1], iota16[0:R, 0:1])
                nc.vector.tensor_add(nt[:, 5:6], b2rep[0:R, 0:1], b3rep[0:R, 0:1])
                nc.sync.dma_start(out_d.ap(), nt[:])

            if do_tail:
                # floor(cx*n) for the band pick index (packed 2-col floorize)
                kf12 = _floorize2(nc, sb, v12[:], "kf12")
                # x1 f32 rows (for band values + mask), off critical path
                x1f = sb.tile([R, N1], F32, name="x1f")
                nc.vector.tensor_tensor(x1f[:], xall[0:R, HID:3 * HID],
                                        b1rep[0:R, :], ALU.add)

                # ---- layer 1 kwta + mm2 ----
                consts1 = dict(consts, pass0_cnt=p0cnt)
                masked1, thr1 = _kwta(nc, sb, pst, xb1, x1f[:], kv12[:, 0:1],
                                      kv12[0:R, 0:1], kf12[0:R, 0:1],
                                      N1, consts1, "L1", skip_pass0=True)
                xt1 = _transpose_chunks(nc, sb, pst, masked1, N1, ident, True, "L1")
                px2 = ap.tile([128, HID], F32, tag="acc3", bufs=2, name="px2")
                w2v = w2sb[:].rearrange("p (c w) -> p c w", c=8)
                for ch in range(8):
                    nc.tensor.matmul(px2[:], xt1[ch][:], w2v[:, ch, :],
                                     start=(ch == 0), stop=(ch == 7))
                xb2 = sb.tile([128, HID], BF16, name="xb2")
                nc.vector.tensor_tensor(xb2[:], px2[:], b2rep[:], ALU.add)
                x2f = sb.tile([R, HID], F32, name="x2f")
                nc.vector.tensor_tensor(x2f[:], px2[0:R, :], b2rep[0:R, :], ALU.add)

                # ---- layer 2 kwta + mm3 ----
                masked2, thr2 = _kwta(nc, sb, pst, xb2, x2f[:], kv12[:, 1:2],
                                      kv12[0:R, 1:2], kf12[0:R, 1:2],
                                      HID, consts, "L2")
                xt2 = _transpose_chunks(nc, sb, pst, masked2, HID, ident, True, "L2")
                px3 = ap.tile([128, N3], F32, tag="acc3", bufs=2, name="px3")
                w3v = w3sb[:].rearrange("p (c w) -> p c w", c=4)
                for ch in range(4):
                    for o in range(2):
                        nc.tensor.matmul(px3[:, 512 * o:512 * (o + 1)], xt2[ch][:],
                                         w3v[:, ch, 512 * o:512 * (o + 1)],
                                         start=(ch == 0), stop=(ch == 3))
                xb3 = sb.tile([128, N3], BF16, name="xb3")
                x3f = sb.tile([R, N3], F32, name="x3f")
                nc.vector.tensor_tensor(xb3[:], px3[:], b3rep[:], ALU.add)
                nc.vector.tensor_tensor(x3f[:], px3[0:R, :], b3rep[0:R, :], ALU.add)

                # ---- layer 3 kwta + mm4 ----  (k3 == k1: same kv/kf)
                masked3, thr3 = _kwta(nc, sb, pst, xb3, x3f[:], kv12[:, 0:1],
                                      kv12[0:R, 0:1], kf12[0:R, 0:1],
                                      N3, consts, "L3")
                xt3 = _transpose_chunks(nc, sb, pst, masked3, N3, ident, False, "L3")
                px4 = ap.tile([R, N3], F32, tag="acc3", bufs=2, name="px4")
                w4v = w4sb[:].rearrange("p (c w) -> p c w", c=8)
                for ch in range(8):
                    for o in range(2):
                        nc.tensor.matmul(px4[:, 512 * o:512 * (o + 1)], xt3[ch][:],
                                         w4v[:, ch, 512 * o:512 * (o + 1)],
                                         start=(ch == 0), stop=(ch == 7))
                outsb = sb.tile([R, N3], F32, name="outsb")
                nc.vector.tensor_copy(outsb[:], px4[:])
                nc.sync.dma_start(out_d.ap(), outsb[:])

    nc.compile()
    return nc


def host_prepare(inputs, cfg: Cfg):
    """Build per-core in_maps from the full inputs."""
    B, NC, KT, SW, KSH = cfg.B, cfg.NC, cfg.KT, cfg.SW, cfg.KSH
    f32 = np.float32
    inp = np.asarray(inputs["input"], f32)
    W_c1 = np.asarray(inputs["W_c1"], f32)
    b_c1 = np.asarray(inputs["b_c1"], f32)
    W_c2 = np.asarray(inputs["W_c2"], f32)
    W1 = np.asarray(inputs["W1"], f32)
    b1 = np.asarray(inputs["b1"], f32)
    W2 = np.asarray(inputs["W2"], f32)
    b2 = np.asarray(inputs["b2"], f32)
    W3 = np.asarray(inputs["W3"], f32)
    b3 = np.asarray(inputs["b3"], f32)
    W4 = np.asarray(inputs["W4"], f32)

    xT = np.ascontiguousarray(inp.T)          # [S2, B]
    wc1T = np.ascontiguousarray(W_c1.T)       # [S2, HID]
    w1T = np.ascontiguousarray(W1.T)          # [S2, N1]

    W = widths()
    fracv = ((np.arange(128, dtype=f32) // R + 1.0) / 5.0)[:, None].astype(f32)
    lo0 = np.float32(-16.0)
    bis0 = np.concatenate([
        lo0 + np.float32(W[0]) * fracv,       # probes_0
        lo0 + np.float32(W[1]) * fracv,       # lofrac_0
        np.full((128, 1), lo0, f32),          # lo_0
    ], axis=1)

    import ml_dtypes
    bf16 = ml_dtypes.bfloat16
    consts = {
        "ident": np.eye(128, dtype=f32),
        "bc1rep": np.broadcast_to(b_c1, (128, HID)).copy(),
        "b1rep": np.broadcast_to(b1, (128, N1)).copy(),
        "b2rep": np.broadcast_to(b2, (128, HID)).copy(),
        "b3rep": np.broadcast_to(b3, (128, N3)).copy(),
        "wc2rep": np.broadcast_to(W_c2[0], (128, HID)).copy(),
        "frac": fracv.copy(),
        "bis0": np.ascontiguousarray(bis0),
        "nvec": np.broadcast_to(np.array([N1, HID], f32), (128, 2)).copy(),
        "iota16": np.broadcast_to(np.arange(16, dtype=f32), (R, 16)).copy(),
        "w2t": np.ascontiguousarray(W2.T),
        "w3t": np.ascontiguousarray(W3.T),
        "w4t": np.ascontiguousarray(W4.T),
    }

    in_maps = []
    for c in range(NC):
        sl = slice(c * KSH, (c + 1) * KSH)
        stream = np.concatenate([xT[sl], wc1T[sl], w1T[sl]], axis=1)  # [KSH, SW]
        hi = stream.astype(bf16)
        lo = (stream - hi.astype(f32)).astype(bf16)
        shl = np.stack([hi, lo], axis=1).reshape(KSH, 2, cfg.SW)  # [KSH,2,SW]
        shl = np.ascontiguousarray(shl.reshape(KT, 128, 2, cfg.SW))
        in_maps.append({"stream": shl, **consts})
    return in_maps


_CACHE = {}


def kernel(**inputs) -> np.ndarray:
    cfg = Cfg(S2=inputs["input"].shape[1], B=inputs["input"].shape[0])
    key = (cfg.S2, cfg.B, cfg.NC)
    if key not in _CACHE:
        _CACHE[key] = build_nc(cfg)
    nc = _CACHE[key]
    in_maps = host_prepare(inputs, cfg)
    res = bass_utils.run_bass_kernel_spmd(
        nc, in_maps, core_ids=list(range(cfg.NC)))
    return np.concatenate([res.results[c]["out"] for c in range(cfg.NC)], axis=0)


if __name__ == "__main__":
    rng = np.random.default_rng(0)
    S2, B = 32768, 256
    demo = {
        "input": rng.standard_normal((B, S2), dtype=np.float32),
        "W_c1": rng.standard_normal((HID, S2), dtype=np.float32) / np.sqrt(S2),
        "b_c1": rng.standard_normal(HID).astype(np.float32) / np.sqrt(S2),
        "W_c2": rng.standard_normal((1, HID), dtype=np.float32) / np.sqrt(HID),
        "W1": rng.standard_normal((N1, S2), dtype=np.float32) / np.sqrt(S2),
        "b1": rng.standard_normal(N1).astype(np.float32) / np.sqrt(S2),
        "W2": rng.standard_normal((HID, N1), dtype=np.float32) / np.sqrt(N1),
        "b2": rng.standard_normal(HID).astype(np.float32) / np.sqrt(N1),
        "W3": rng.standard_normal((N3, HID), dtype=np.float32) / np.sqrt(HID),
        "b3": rng.standard_normal(N3).astype(np.float32) / np.sqrt(HID),
        "W4": rng.standard_normal((N3, N3), dtype=np.float32) / np.sqrt(N3),
    }
    out = kernel(**demo)
    print(out.shape, out.dtype, np.abs(out).max())
